# revision 25
# baseline (speedup 1.0000x reference)
"""MAE self-attention (sparse_attention) Trainium2 Bass kernel, v2.

Sharding: 8 cores = batch(2) x head-groups(4 groups of 3 heads).

Key ideas vs v1:
  - Key compaction on host: mlm-masked keys (~15%) are dropped before the
    kernel runs; keys = [embx ; valid hidden positions ; zero pad] padded to
    NT*128 = 1792 (vs 2176).  All engines do ~18% less work.
  - Scores/exp/pv identical pipeline per (head, q-half, key-tile) step, but
    pt (exp of scores) is written as fp8e4m3 and the pv matmul runs in
    DoubleRow mode over key-tile PAIRS (contraction 256), halving PE rows.
  - The enhanced-decoding diagonal (query q must not see its own key) is
    applied by a small extra matmul accumulating -1e5 * indicator into the
    score PSUM before exp: lhsT = -1e5*I[128], rhs = host-built indicator
    D[key_lane, tile, q - wstart(tile)] over a fixed 512-wide q window.
  - Projections all bf16 (halves the xT DMA); the v bias is folded in as an
    extra contraction row (ones x bias_row), so the PSUM->fp8 conversion of
    v is a single copy.
  - Pad keys are killed by a per-partition bias of -10000 on the last key
    tile's ACT instruction (exp -> exactly 0), so they contribute nothing to
    numerator or denominator.  Host divides by the denominator row (row 64)
    after gathering.
"""

import ml_dtypes
import numpy as np

import concourse.bacc as bacc
import concourse.bass as bass  # noqa: F401
import concourse.mybir as mybir
import concourse.tile as tile
from concourse.bass_utils import run_bass_kernel_spmd

F32 = mybir.dt.float32
F32R = mybir.dt.float32r
BF16 = mybir.dt.bfloat16
FP8 = mybir.dt.float8e4

B = 2
S = 2048          # queries
HID = 768
H = 12
D = 64
G = 3             # heads per core
NCORE = 8
NT = 14           # key tiles after compaction (1737/1743 true keys)
SK = NT * 128     # 1792 padded keys
NPAIR = NT // 2   # 7 key-tile pairs for DoubleRow pv
KC = HID // 128   # 6 contraction chunks
NEG = -10000.0
SCALE = 0.125     # D ** -0.5
# exp() is emitted as exp(score*SCALE + EBIAS) = exp(score*SCALE)/16 so the
# fp8e4m3 pt values stay under 448 (max observed exp is ~1542).  The factor
# cancels in the host-side numerator/denominator division.
EBIAS = -2.772588722239781  # -ln(16)
DW = 512          # diag-mask q window width per key tile
WCOLS = 4 * D + G * D      # [k_h0|k_h1|k_h2|k_h2] 256 + 192 v cols

Exp = mybir.ActivationFunctionType.Exp

USE_FP8_PV = False


def _wstart(t):
    return max(0, t * 128 - 64)


def _build_nc(ablate=(), reps=1):
    nc = bacc.Bacc(None, target_bir_lowering=False)

    xT_d = nc.dram_tensor("xT", [HID, SK], BF16, kind="ExternalInput")
    qT_d = nc.dram_tensor("qT", [G * D, S], BF16, kind="ExternalInput")
    w_d = nc.dram_tensor("W", [HID, WCOLS], BF16, kind="ExternalInput")
    bv_d = nc.dram_tensor("bv", [128, G * D], BF16, kind="ExternalInput")
    bk_d = nc.dram_tensor("bk", [128, 2], F32, kind="ExternalInput")
    kb_d = nc.dram_tensor("kb", [128, 1], F32, kind="ExternalInput")
    dm_d = nc.dram_tensor("Dm", [128, NT, DW], BF16, kind="ExternalInput")
    out_d = nc.dram_tensor("outT", [G, D + 1, S], F32, kind="ExternalOutput")

    with tile.TileContext(nc) as tc:
        with (
            tc.tile_pool(name="const", bufs=1) as cpool,
            tc.tile_pool(name="work", bufs=3) as wpool,
            tc.tile_pool(name="ovec", bufs=2) as opool,
            tc.tile_pool(name="psA", bufs=3, space="PSUM") as psa,
            tc.tile_pool(name="psV", bufs=1, space="PSUM") as psv,
        ):
            xT_sb = cpool.tile([128, KC, SK], BF16)
            w_sb = cpool.tile([128, KC, WCOLS], BF16)
            qT_sb = cpool.tile([128, G, S], BF16)
            bv_sb = cpool.tile([128, G * D], BF16)
            bk_sb = cpool.tile([128, 2], F32)
            kb_sb = cpool.tile([128, 1], F32)
            dm_sb = cpool.tile([128, NT, DW], BF16)
            kT_sb = cpool.tile([128, 2, SK], BF16)
            v_sb = cpool.tile([128, NT, G, D + 1], BF16)

            eb_sb = cpool.tile([128, 1], F32)
            nc.vector.memset(eb_sb, EBIAS)
            # ones column of v' (gives the softmax denominator in pv row 64)
            nc.vector.memset(v_sb[:, :, :, D : D + 1], 1.0)

            # Loads, criticality-ordered across the two HWDGE queues so the
            # first scores/exp/mask can start ~7us in:
            #   SP : bk | xT keys 0:1024 | Dm | kb bv | qT h1/h2
            #   ACT: W | qT h0 (both dup halves) | xT keys 1024:1792
            xTr = xT_d.rearrange("(c p) k -> p c k", p=128)
            wr = w_d.rearrange("(c p) n -> p c n", p=128)
            qTr = qT_d.rearrange("(h p) s -> p h s", p=D)
            nc.sync.dma_start(out=bk_sb, in_=bk_d[:, :])
            nc.sync.dma_start(out=xT_sb[:, :, 0:1024], in_=xTr[:, :, 0:1024])
            nc.sync.dma_start(out=dm_sb[:, 0:10, :], in_=dm_d[:, 0:10, :])
            nc.sync.dma_start(out=dm_sb[:, 10:NT, :], in_=dm_d[:, 10:NT, :])
            nc.sync.dma_start(out=kb_sb, in_=kb_d[:, :])
            nc.sync.dma_start(out=bv_sb, in_=bv_d[:, :])
            nc.sync.dma_start(out=qT_sb[0:D, 1:G, :], in_=qTr[:, 1:G, :])
            nc.sync.dma_start(out=qT_sb[D:128, 1:G, :], in_=qTr[:, 1:G, :])
            nc.scalar.dma_start(out=w_sb, in_=wr)
            nc.scalar.dma_start(out=qT_sb[0:D, 0:1, :], in_=qTr[:, 0:1, :])
            nc.scalar.dma_start(out=qT_sb[D:128, 0:1, :], in_=qTr[:, 0:1, :])
            nc.scalar.dma_start(out=xT_sb[:, :, 1024:SK], in_=xTr[:, :, 1024:SK])

            # ---- kv projection emitters (ride the step stream in small
            # chunks so PE never bursts long enough to starve ACT) ----
            KT_CHUNKS = [(i * 224, 224) for i in range(8)]

            def proj_k_chunk(blk, c0, csz):
                ps = psa.tile([128, 1024], F32, tag="ps")
                for kc in range(KC):
                    nc.tensor.matmul(
                        ps[:, 0:csz],
                        w_sb[:, kc, 128 * blk : 128 * blk + 128],
                        xT_sb[:, kc, c0 : c0 + csz],
                        start=(kc == 0),
                        stop=(kc == KC - 1),
                    )
                nc.vector.tensor_scalar_add(
                    kT_sb[:, blk, c0 : c0 + csz],
                    ps[:, 0:csz],
                    bk_sb[:, blk : blk + 1],
                )

            def proj_v_tile(t):
                ps = psa.tile([128, 1024], F32, tag="ps")
                for kc in range(KC):
                    nc.tensor.matmul(
                        ps[:, 0 : G * D],
                        xT_sb[:, kc, t * 128 : (t + 1) * 128],
                        w_sb[:, kc, 4 * D : WCOLS],
                        start=(kc == 0),
                        stop=(kc == KC - 1),
                    )
                nc.vector.tensor_add(
                    v_sb[:, t, :, 0:D],
                    ps[:, 0 : G * D].rearrange("p (h d) -> p h d", h=G),
                    bv_sb.rearrange("p (h d) -> p h d", h=G),
                )

            # ---- attention: one global step stream over all reps ----
            steps = [(h, half, t) for h in range(G) for half in range(2)
                     for t in range(NT)]
            n_steps = len(steps)
            total_steps = reps * n_steps
            pv_tiles = {}

            def emit_scores(gi):
                h, half, t = steps[gi % n_steps]
                q0 = half * 1024
                ps = psa.tile([128, 1024], F32, tag="ps")
                # h0: block 0 lanes 0:64; h1: block 0 lanes 64:128 (h0/h1
                # stacked, no dup); h2: block 1, dup'd on both halves so its
                # two q-chunks can row-pack on hardware.
                if h == 0:
                    blk, po = 0, (0, 0)
                elif h == 1:
                    blk, po = 0, (D, D)
                else:
                    blk, po = 1, (0, D)
                for ci, p0 in enumerate(po):
                    nc.tensor.matmul(
                        ps[:, ci * 512 : ci * 512 + 512],
                        kT_sb[p0 : p0 + D, blk, t * 128 : (t + 1) * 128],
                        qT_sb[p0 : p0 + D, h, q0 + ci * 512 : q0 + ci * 512 + 512],
                        start=True,
                        stop=True,
                        tile_position=(p0, 0),
                    )
                return ps

            def emit_exp(gi, ps):
                h, half, t = steps[gi % n_steps]
                q0 = half * 1024
                pt = wpool.tile([128, 1024], BF16, tag="pt",
                                name=f"pt_{gi}")
                bias = kb_sb if t == NT - 1 else eb_sb
                nc.scalar.activation(
                    pt, ps, Exp, bias=bias[:, 0:1], scale=SCALE
                )
                # zero the diag band: pt *= (1 - indicator) over the fixed
                # 512-wide window of tile t that overlaps this half
                ws = _wstart(t)
                a = max(ws, q0)
                b = min(ws + DW, q0 + 1024)
                if a < b:
                    nc.vector.tensor_mul(
                        pt[:, a - q0 : b - q0],
                        pt[:, a - q0 : b - q0],
                        dm_sb[:, t, a - ws : b - ws],
                    )
                return pt

            def emit_pv(gi, pt):
                h, half, t = steps[gi % n_steps]
                q0 = half * 1024
                if t == 0:
                    pv_tiles[(h, half)] = psv.tile(
                        [D + 1, 1024], F32, tag="pv", name=f"pv_{gi}",
                    )
                pv = pv_tiles[(h, half)]
                for qc in (0, 512):
                    nc.tensor.matmul(
                        pv[:, qc : qc + 512],
                        v_sb[:, t, h, :],
                        pt[:, qc : qc + 512],
                        start=(t == 0),
                        stop=(t == NT - 1),
                    )
                if t == NT - 1:
                    ov = opool.tile([D + 1, 1024], F32, tag="ov")
                    nc.vector.tensor_copy(ov, pv)
                    nc.sync.dma_start(out=out_d[h, :, q0 : q0 + 1024], in_=ov)

            # Projection placement: rep 0's kT block 0 and first v tiles are
            # a true prolog (overlapped with the input DMAs); everything else
            # rides the step stream, scheduled so the next rep's projections
            # land in the current rep's late steps after their last reader.
            PRE_V = 2
            for c0, csz in KT_CHUNKS[:5]:
                proj_k_chunk(0, c0, csz)
            for t in range(PRE_V):
                proj_v_tile(t)

            work = []
            for r in range(reps):
                base = r * n_steps
                if r == 0:
                    for t in range(PRE_V, NT):
                        work.append((t, lambda t=t: proj_v_tile(t)))
                    for ci, (c0, csz) in enumerate(KT_CHUNKS[5:]):
                        work.append((5 + ci, lambda c0=c0, csz=csz:
                                     proj_k_chunk(0, c0, csz)))
                else:
                    # vproj(t) last read by rep r-1 at its step 70+t
                    for t in range(NT):
                        work.append((base - n_steps + 71 + t,
                                     lambda t=t: proj_v_tile(t)))
                    # kT block 0 last read by rep r-1 at its step 55
                    for ci, (c0, csz) in enumerate(KT_CHUNKS):
                        work.append((base - n_steps + 58 + 2 * ci,
                                     lambda c0=c0, csz=csz:
                                     proj_k_chunk(0, c0, csz)))
                # kT block 1 (head 2) needed from step base+56
                for ci, (c0, csz) in enumerate(KT_CHUNKS):
                    work.append((base + 20 + 2 * ci,
                                 lambda c0=c0, csz=csz:
                                 proj_k_chunk(1, c0, csz)))
            work.sort(key=lambda x: x[0])

            LOOKAHEAD = 2
            prev = {}
            for i in range(total_steps + LOOKAHEAD):
                if i < total_steps:
                    prev[i] = emit_scores(i)
                    while work and work[0][0] <= i:
                        work.pop(0)[1]()
                j = i - LOOKAHEAD
                if j >= 0:
                    pt = emit_exp(j, prev.pop(j))
                    emit_pv(j, pt)
                    if work and (j % 2 == 1):
                        work.pop(0)[1]()
            while work:
                work.pop(0)[1]()

    nc.finalize()
    return nc


_NC = None


def _get_nc():
    global _NC
    if _NC is None:
        _NC = _build_nc()
    return _NC


def _host_prep(hidden_states, embx, expanded_embx, Wkv_w, Wkv_b,
               attention_mask, mlm_mask):
    hs = np.asarray(hidden_states, np.float32)
    ex = np.asarray(embx, np.float32)
    qx = np.asarray(expanded_embx, np.float32)
    w = np.asarray(Wkv_w, np.float32)
    bb = np.asarray(Wkv_b, np.float32)
    am = np.asarray(attention_mask).astype(bool)
    mm = np.asarray(mlm_mask).astype(bool)

    valid = am & ~mm                                    # (B, S)

    # Per batch: compacted key order, xT, diag indicator, pad bias.
    xT_b, dm_b, kb_b = [], [], []
    for b in range(B):
        perm = np.nonzero(valid[b])[0]                  # orig positions
        n_b = 1 + len(perm)
        assert n_b <= SK, f"batch {b}: {n_b} keys > {SK}"
        x = np.zeros((SK, HID), np.float32)
        x[0] = ex[b, 0]
        x[1:n_b] = hs[b, perm]
        xT_b.append(np.ascontiguousarray(x.T.astype(ml_dtypes.bfloat16)))

        dm = np.ones((128, NT, DW), np.float32)
        for j in range(1, n_b):
            q = int(perm[j - 1])                        # query excluded by col j
            t, l = divmod(j, 128)
            i = q - _wstart(t)
            assert 0 <= i < DW, f"diag window miss: b{b} j{j} q{q} t{t} i{i}"
            dm[l, t, i] = 0.0
        dm_b.append(dm.astype(ml_dtypes.bfloat16))

        kb = np.full((128, 1), EBIAS, np.float32)
        thr = n_b - (NT - 1) * 128                      # valid lanes in last tile
        kb[max(0, thr):, 0] = NEG
        kb_b.append(kb)

    in_maps = []
    for c in range(NCORE):
        b, g = divmod(c, 4)
        k_cols = slice(192 * g, 192 * g + 192)
        v_cols = slice(768 + 192 * g, 768 + 192 * g + 192)
        w_h = [w[:, 192 * g + 64 * h : 192 * g + 64 * h + 64] for h in range(G)]
        wparts = [w_h[0], w_h[1], w_h[2], w_h[2], w[:, v_cols]]
        wg = np.concatenate(wparts, axis=1).astype(ml_dtypes.bfloat16)
        bk1 = bb[k_cols].reshape(G, D).T                # (64, 3): cols h0,h1,h2
        bk = np.ascontiguousarray(np.stack(
            [np.concatenate([bk1[:, 0], bk1[:, 1]]),
             np.concatenate([bk1[:, 2], bk1[:, 2]])], axis=1))  # (128, 2)
        bv = np.ascontiguousarray(np.broadcast_to(
            bb[v_cols].astype(ml_dtypes.bfloat16), (128, G * D)))
        qtg = np.ascontiguousarray(
            qx[b][:, k_cols].T.astype(ml_dtypes.bfloat16)
        )                                               # (192, 2048) bf16
        in_maps.append(
            dict(xT=xT_b[b], qT=qtg, W=np.ascontiguousarray(wg), bv=bv,
                 bk=bk, kb=kb_b[b], Dm=dm_b[b])
        )
    return in_maps


def _host_post(results):
    out = np.empty((B, S, HID), np.float32)
    for c in range(NCORE):
        b, g = divmod(c, 4)
        ot = results[c]["outT"]                         # (3, 65, 2048)
        o = ot[:, :D, :] / ot[:, D : D + 1, :]          # (3, 64, 2048)
        out[b, :, 192 * g : 192 * g + 192] = (
            o.transpose(2, 0, 1).reshape(S, G * D)
        )
    return out


def kernel(hidden_states, embx, expanded_embx, Wkv_w, Wkv_b,
           attention_mask, mlm_mask):
    in_maps = _host_prep(hidden_states, embx, expanded_embx, Wkv_w, Wkv_b,
                         attention_mask, mlm_mask)
    nc = _get_nc()
    res = run_bass_kernel_spmd(nc, in_maps, list(range(NCORE)))
    return _host_post(res.results)
